# revision 8
# baseline (speedup 1.0000x reference)
"""Trainium2 Bass kernel for a contextual loss (cosine-distance softmin loss).

Math (per batch b):
  mu_c      = mean_n Y[b,c,n]
  xc = X-mu, yc = Y-mu                      (centered, [C,N])
  G[i,j]    = <xc_i, yc_j>                  (K=C=64 matmul, bf16 inputs)
  rb[i,j]   = G[i,j] * ry_j                 (ry = 1/||yc_j||; folded into the
                                             PSUM->SBUF copy+max pass)
  pmax_i    = max_j rb[i,j]                 (so smax_i = rx_i * pmax_i)
  aa_i      = rx5_i / (1.001 - 5*rx5_i*pmax_i);  bb_i = -aa_i*pmax_i
  S_i       = sum_j exp(aa_i*rb[i,j] + bb_i)
  loss_b    = -log(mean_i 1/S_i)

Sharding: 8 cores = 4 batches x 2 row-halves. Each core gets its full-batch
Y [64,4096] and its half of X's columns [64,2048], computes S_i for its 2048
rows, returns S as [128,16] (partition p, chunk k <-> row k*128+p). Host
reduces to the [4] loss.

Per-engine pipeline per 128-row chunk (DVE-bound):
  PE   : 8 bf16 matmuls (K=64, N=512) -> PSUM [128,2048] x2
  DVE  : tensor_tensor_reduce fuses (G * ry_row) -> SBUF rb copy (bf16)
         with a running row-max (accum); per-row aa/bb stats batched in
         groups of 4 chunks as [128,4] ops
  ACT  : one exp(aa*rb + bb) per chunk over [128,4096] with accumulated
         row-sum (accum_out) -> S
"""

import math

import numpy as np

import concourse.bacc as bacc
import concourse.mybir as mybir
from concourse.bass_utils import run_bass_kernel_spmd
from concourse.dve_ops import TENSOR_MASK_REDUCE
from concourse.mybir import ActivationFunctionType as AF, AluOpType as OP, AxisListType
from concourse.tile import TileContext

F32 = mybir.dt.float32
FP16 = mybir.dt.float16

B, C, N = 4, 64, 4096          # batch, channels, spatial (64*64)
NX = N // 2                    # rows per core (half batch)
CH = NX // 128                 # 16 chunks of 128 rows
HALF = N // 2                  # column half per PSUM buffer
GRP = 4                        # chunks per stats batch
H_BAND = 5.0
EPS_MIN = 1e-3
LN02 = math.log(0.2)           # fold the 1/H into rx via exp(... + ln(1/H))
NEG_BIG = -3.0e38

_NC_CACHE = {}


def build_nc():
    nc = bacc.Bacc("TRN2", target_bir_lowering=False, debug=False, num_devices=8)
    x_d = nc.dram_tensor("Xh", [C, NX], F32, kind="ExternalInput")
    y_d = nc.dram_tensor("Yb", [C, N], F32, kind="ExternalInput")
    out_d = nc.dram_tensor("out", [128, CH], F32, kind="ExternalOutput")

    with TileContext(nc) as tc:
        with (
            tc.tile_pool(name="persist", bufs=1) as persist,
            tc.tile_pool(name="mm", bufs=2, space="PSUM") as mmpool,
            tc.tile_pool(name="rb", bufs=2 * GRP) as rbpool,
            tc.tile_pool(name="lnt", bufs=2) as lnpool,
            tc.tile_pool(name="dummy", bufs=2) as dummypool,
        ):
            # ---------------- load inputs ----------------
            x_sb = persist.tile([C, NX], F32)
            nc.sync.dma_start(out=x_sb[:], in_=x_d[:])
            y_sb = persist.tile([C, N], F32)
            nc.sync.dma_start(out=y_sb[:, 0:HALF], in_=y_d[:, 0:HALF])
            nc.sync.dma_start(out=y_sb[:, HALF:N], in_=y_d[:, HALF:N])

            onesb = persist.tile([C, 128], FP16)
            nc.vector.memset(onesb[:], 1.0)
            ones2b = persist.tile([C, 2], FP16)
            nc.vector.memset(ones2b[:], 1.0)
            ln02 = persist.tile([128, 1], F32)
            nc.vector.memset(ln02[:], LN02)
            c3big = persist.tile([128, 1], F32)
            nc.vector.memset(c3big[:], 1.0e9)

            # ---------------- Y spatial mean ----------------
            r2 = persist.tile([C, 2], F32)
            nc.vector.reduce_sum(out=r2[:, 0:1], in_=y_sb[:, 0:HALF], axis=AxisListType.X)
            nc.vector.reduce_sum(out=r2[:, 1:2], in_=y_sb[:, HALF:N], axis=AxisListType.X)
            musum = persist.tile([C, 1], F32)
            nc.vector.tensor_tensor(musum[:], r2[:, 0:1], r2[:, 1:2], OP.add)
            mu = persist.tile([C, 1], F32)
            nc.vector.tensor_scalar_mul(mu[:], musum[:], 1.0 / N)

            # ---------------- center + squares ----------------
            # ycen kept f32 so yhat = ycen*ry is single-rounded to fp16;
            # squares single-rounded on ACT via Square(y - mu).
            negmu = persist.tile([C, 1], F32)
            nc.vector.tensor_scalar_mul(negmu[:], mu[:], -1.0)
            ycen32 = persist.tile([C, N], F32)
            xcenb = persist.tile([C, NX], FP16)
            ysqb = persist.tile([C, N], FP16)
            xsqb = persist.tile([C, NX], FP16)
            nc.vector.tensor_scalar(ycen32[:, 0:HALF], y_sb[:, 0:HALF], mu[:], None, OP.subtract)
            nc.vector.tensor_scalar(xcenb[:], x_sb[:], mu[:], None, OP.subtract)
            nc.scalar.activation(ysqb[:, 0:HALF], y_sb[:, 0:HALF], AF.Square, bias=negmu[:])
            nc.scalar.activation(xsqb[:], x_sb[:], AF.Square, bias=negmu[:])
            nc.vector.tensor_scalar(ycen32[:, HALF:N], y_sb[:, HALF:N], mu[:], None, OP.subtract)
            nc.scalar.activation(ysqb[:, HALF:N], y_sb[:, HALF:N], AF.Square, bias=negmu[:])

            # ---------------- column norms of Y -> ry_b [64, N] ------------
            # ones[64,64].T @ ysq[64,512] = column sums of ysq, replicated
            # down 64 partitions.  ry = 1/sqrt(ny2) done as exp(-.5*ln).
            ry_b = persist.tile([C, N], F32)
            ny_ps = []
            for h in range(2):
                ps = mmpool.tile([C, HALF], F32, tag="mm")
                for j in range(4):
                    c0 = h * HALF + j * 512
                    nc.tensor.matmul(
                        ps[:, j * 512:(j + 1) * 512],
                        lhsT=onesb[:, 0:C],
                        rhs=ysqb[:, c0:c0 + 512],
                        start=True, stop=True,
                    )
                ny_ps.append(ps)

            # ---------------- row norms of X -> nx2 [128, 2*CH] -------------
            nxps = mmpool.tile([128, 2 * CH], F32, tag="mm")
            for k in range(CH):
                nc.tensor.matmul(
                    nxps[:, 2 * k:2 * k + 2],
                    lhsT=xsqb[:, k * 128:(k + 1) * 128],
                    rhs=ones2b[:],
                    start=True, stop=True,
                )

            # ACT: all Lns grouped, then all Exps (one act-table set each)
            tlns = []
            for h in range(2):
                tln = lnpool.tile([C, HALF], F32, tag="lnt")
                nc.scalar.activation(tln[:], ny_ps[h][:], AF.Ln)
                tlns.append(tln)
            tn = persist.tile([128, CH], F32)
            nc.scalar.activation(
                tn[:], nxps[:].rearrange("p (k two) -> p k two", two=2)[:, :, 0], AF.Ln
            )
            for h in range(2):
                nc.scalar.activation(
                    ry_b[:, h * HALF:(h + 1) * HALF], tlns[h][:], AF.Exp, scale=-0.5
                )
            rx5 = persist.tile([128, CH], F32)
            nc.scalar.activation(rx5[:], tn[:], AF.Exp, bias=ln02[:], scale=-0.5)

            # yhat = ycen * ry  (f32 x f32 -> fp16, single rounding)
            yhatb = persist.tile([C, N], FP16)
            for h in range(2):
                s_ = slice(h * HALF, (h + 1) * HALF)
                nc.vector.tensor_tensor(yhatb[:, s_], ycen32[:, s_], ry_b[:, s_], OP.mult)

            rx5x5 = persist.tile([128, CH], F32)
            nc.vector.tensor_scalar_mul(rx5x5[:], rx5[:], H_BAND)
            nrx5 = persist.tile([128, CH], F32)
            nc.vector.tensor_scalar_mul(nrx5[:], rx5[:], -1.0)

            # ---------------- main loop ----------------
            pmtmp = persist.tile([128, CH], F32)
            pmfin = persist.tile([128, CH], F32)
            aa = persist.tile([128, CH], F32)
            bb = persist.tile([128, CH], F32)
            smax = persist.tile([128, CH], F32)
            den = persist.tile([128, CH], F32)
            rec = persist.tile([128, CH], F32)
            aan = persist.tile([128, CH], F32)
            ssums = persist.tile([128, CH], F32)
            rbs = {}

            for k in range(CH):
                lhs = xcenb[:, k * 128:(k + 1) * 128]
                rb = rbpool.tile([128, N], FP16, tag="rb")
                rbs[k] = rb
                for h in range(2):
                    ps = mmpool.tile([128, HALF], F32, tag="mm")
                    for j in range(4):
                        c0 = h * HALF + j * 512
                        nc.tensor.matmul(
                            ps[:, j * 512:(j + 1) * 512],
                            lhsT=lhs,
                            rhs=yhatb[:, c0:c0 + 512],
                            start=True, stop=True,
                        )
                    # rb = copy(ps); accum = running row-max
                    # (custom-DVE mask-reduce with an all-pass window)
                    init = NEG_BIG if h == 0 else pmtmp[:, k:k + 1]
                    acc = (pmtmp if h == 0 else pmfin)[:, k:k + 1]
                    nc.vector._custom_dve(
                        TENSOR_MASK_REDUCE,
                        out=rb[:, h * HALF:(h + 1) * HALF],
                        in0=ps[:],
                        in1=c3big[:],
                        s0=0.0,
                        s1=init,
                        imm2=1.0,
                        accum_out=acc,
                    )

                if k % GRP == GRP - 1:
                    # batched per-row stats for chunks g0..k: [128, GRP] ops
                    g0 = k - GRP + 1
                    s_ = slice(g0, k + 1)
                    nc.vector.tensor_tensor(smax[:, s_], pmfin[:, s_], rx5x5[:, s_], OP.mult)
                    nc.vector.tensor_scalar(
                        den[:, s_], smax[:, s_], -1.0, 1.0 + EPS_MIN, OP.mult, OP.add
                    )
                    nc.vector.reciprocal(rec[:, s_], den[:, s_])
                    nc.vector.tensor_tensor(aan[:, s_], rec[:, s_], nrx5[:, s_], OP.mult)
                    nc.vector.tensor_tensor(bb[:, s_], aan[:, s_], pmfin[:, s_], OP.mult)
                    nc.vector.tensor_scalar_mul(aa[:, s_], aan[:, s_], -1.0)

                    for kk in range(g0, k + 1):
                        dummy = dummypool.tile([128, N], FP16, tag="dummy")
                        nc.scalar.activation(
                            dummy[:],
                            rbs.pop(kk)[:],
                            AF.Exp,
                            bias=bb[:, kk:kk + 1],
                            scale=aa[:, kk:kk + 1],
                            accum_out=ssums[:, kk:kk + 1],
                        )

            # ---------------- finalize ----------------
            nc.sync.dma_start(out=out_d[:], in_=ssums[:])

    nc.compile()
    return nc


def _get_nc():
    if "nc" not in _NC_CACHE:
        _NC_CACHE["nc"] = build_nc()
    return _NC_CACHE["nc"]


def make_in_maps(X_features, Y_features):
    X = np.ascontiguousarray(np.asarray(X_features, np.float32).reshape(B, C, N))
    Y = np.ascontiguousarray(np.asarray(Y_features, np.float32).reshape(B, C, N))
    in_maps = []
    for c in range(8):
        b, h = divmod(c, 2)
        in_maps.append({
            "Xh": np.ascontiguousarray(X[b, :, h * NX:(h + 1) * NX]),
            "Yb": Y[b],
        })
    return in_maps


def combine(results):
    """results: list of 8 dicts with 'out' [128, CH] = S per row."""
    out = np.empty(B, np.float32)
    for b in range(B):
        tot = 0.0
        for h in range(2):
            s = results[2 * b + h]["out"].astype(np.float64)
            tot += (1.0 / s).sum()
        out[b] = -np.log(tot / N)
    return out


def kernel(X_features, Y_features):
    nc = _get_nc()
    in_maps = make_in_maps(X_features, Y_features)
    res = run_bass_kernel_spmd(nc, in_maps, core_ids=list(range(8)))
    return combine(res.results)


if __name__ == "__main__":
    rng = np.random.default_rng(0)
    X = rng.standard_normal((B, C, 64, 64)).astype(np.float32)
    Y = rng.standard_normal((B, C, 64, 64)).astype(np.float32)
    print(kernel(X_features=X, Y_features=Y))


# revision 9
# speedup vs baseline: 1.0539x; 1.0539x over previous
"""Trainium2 Bass kernel for a contextual loss (cosine-distance softmin loss).

Math (per batch b):
  mu_c      = mean_n Y[b,c,n]
  xc = X-mu, yc = Y-mu                      (centered, [C,N])
  G[i,j]    = <xc_i, yc_j>                  (K=C=64 matmul, bf16 inputs)
  rb[i,j]   = G[i,j] * ry_j                 (ry = 1/||yc_j||; folded into the
                                             PSUM->SBUF copy+max pass)
  pmax_i    = max_j rb[i,j]                 (so smax_i = rx_i * pmax_i)
  aa_i      = rx5_i / (1.001 - 5*rx5_i*pmax_i);  bb_i = -aa_i*pmax_i
  S_i       = sum_j exp(aa_i*rb[i,j] + bb_i)
  loss_b    = -log(mean_i 1/S_i)

Sharding: 8 cores = 4 batches x 2 row-halves. Each core gets its full-batch
Y [64,4096] and its half of X's columns [64,2048], computes S_i for its 2048
rows, returns S as [128,16] (partition p, chunk k <-> row k*128+p). Host
reduces to the [4] loss.

Per-engine pipeline per 128-row chunk (DVE-bound):
  PE   : 8 bf16 matmuls (K=64, N=512) -> PSUM [128,2048] x2
  DVE  : tensor_tensor_reduce fuses (G * ry_row) -> SBUF rb copy (bf16)
         with a running row-max (accum); per-row aa/bb stats batched in
         groups of 4 chunks as [128,4] ops
  ACT  : one exp(aa*rb + bb) per chunk over [128,4096] with accumulated
         row-sum (accum_out) -> S
"""

import math

import numpy as np

import concourse.bacc as bacc
import concourse.mybir as mybir
from concourse.bass_utils import run_bass_kernel_spmd
from concourse.hw_specs import get_activation_tables
from concourse.dve_ops import TENSOR_MASK_REDUCE
from concourse.mybir import ActivationFunctionType as AF, AluOpType as OP, AxisListType
from concourse.tile import TileContext

F32 = mybir.dt.float32
FP16 = mybir.dt.float16

B, C, N = 4, 64, 4096          # batch, channels, spatial (64*64)
NX = N // 2                    # rows per core (half batch)
CH = NX // 128                 # 16 chunks of 128 rows
HALF = N // 2                  # column half per PSUM buffer
GRP = 4                        # chunks per stats batch
H_BAND = 5.0
EPS_MIN = 1e-3
LN02 = math.log(0.2)           # fold the 1/H into rx via exp(... + ln(1/H))
NEG_BIG = -3.0e38

_NC_CACHE = {}


def _force_one_act_table(arch):
    # ln/exp/square all live in natural_log_exp_and_others; strip them from
    # every other set so the table-load pass can't thrash between sets.
    tabs = get_activation_tables(arch)
    for name, fns in tabs.items():
        if name != "natural_log_exp_and_others":
            fns.discard(AF.Ln)
            fns.discard(AF.Exp)
            fns.discard(AF.Square)


def build_nc():
    nc = bacc.Bacc("TRN2", target_bir_lowering=False, debug=False, num_devices=8)
    _force_one_act_table(nc.m.arch)
    x_d = nc.dram_tensor("Xh", [C, NX], F32, kind="ExternalInput")
    y_d = nc.dram_tensor("Yb", [C, N], F32, kind="ExternalInput")
    out_d = nc.dram_tensor("out", [128, CH], F32, kind="ExternalOutput")

    with TileContext(nc) as tc:
        with (
            tc.tile_pool(name="persist", bufs=1) as persist,
            tc.tile_pool(name="mm", bufs=2, space="PSUM") as mmpool,
            tc.tile_pool(name="rb", bufs=2 * GRP) as rbpool,
            tc.tile_pool(name="lnt", bufs=2) as lnpool,
            tc.tile_pool(name="dummy", bufs=2) as dummypool,
        ):
            # ---------------- load inputs (strip DMAs) ----------------
            y_sb = persist.tile([C, N], F32)
            QS = N // 4
            for q in range(4):
                nc.sync.dma_start(out=y_sb[:, q * QS:(q + 1) * QS], in_=y_d[:, q * QS:(q + 1) * QS])
            x_sb = persist.tile([C, NX], F32)
            nc.sync.dma_start(out=x_sb[:, 0:NX // 2], in_=x_d[:, 0:NX // 2])
            nc.sync.dma_start(out=x_sb[:, NX // 2:NX], in_=x_d[:, NX // 2:NX])

            onesb = persist.tile([C, 128], FP16)
            nc.vector.memset(onesb[:], 1.0)
            ones2b = persist.tile([C, 2], FP16)
            nc.vector.memset(ones2b[:], 1.0)
            ln02 = persist.tile([128, 1], F32)
            nc.vector.memset(ln02[:], LN02)
            c3big = persist.tile([128, 1], F32)
            nc.vector.memset(c3big[:], 1.0e9)

            # ---------------- Y spatial mean ----------------
            r4 = persist.tile([C, 4], F32)
            for q in range(4):
                nc.vector.reduce_sum(
                    out=r4[:, q:q + 1], in_=y_sb[:, q * QS:(q + 1) * QS], axis=AxisListType.X
                )
            mu = persist.tile([C, 1], F32)
            nc.vector.reduce_sum(out=mu[:], in_=r4[:], axis=AxisListType.X)
            nc.vector.tensor_scalar_mul(mu[:], mu[:], 1.0 / N)

            # ---------------- center + squares ----------------
            # ycen kept f32 so yhat = ycen*ry is single-rounded to fp16;
            # squares single-rounded on ACT via Square(y - mu).
            negmu = persist.tile([C, 1], F32)
            nc.vector.tensor_scalar_mul(negmu[:], mu[:], -1.0)
            ycen16 = persist.tile([C, N], FP16)
            xcenb = persist.tile([C, NX], FP16)
            ysqb = persist.tile([C, N], FP16)
            xsqb = persist.tile([C, NX], FP16)
            nc.vector.tensor_scalar(ycen16[:, 0:HALF], y_sb[:, 0:HALF], mu[:], None, OP.subtract)
            nc.vector.tensor_scalar(xcenb[:], x_sb[:], mu[:], None, OP.subtract)
            nc.scalar.activation(ysqb[:, 0:HALF], y_sb[:, 0:HALF], AF.Square, bias=negmu[:])
            nc.scalar.activation(xsqb[:], x_sb[:], AF.Square, bias=negmu[:])
            nc.vector.tensor_scalar(ycen16[:, HALF:N], y_sb[:, HALF:N], mu[:], None, OP.subtract)
            nc.scalar.activation(ysqb[:, HALF:N], y_sb[:, HALF:N], AF.Square, bias=negmu[:])

            # ---------------- column norms of Y -> ry_b [64, N] ------------
            # ones[64,64].T @ ysq[64,512] = column sums of ysq, replicated
            # down 64 partitions.  ry = 1/sqrt(ny2) done as exp(-.5*ln).
            ry_b = persist.tile([C, N], FP16)
            ny_ps = []
            for h in range(2):
                ps = mmpool.tile([C, HALF], F32, tag="mm")
                for j in range(4):
                    c0 = h * HALF + j * 512
                    nc.tensor.matmul(
                        ps[:, j * 512:(j + 1) * 512],
                        lhsT=onesb[:, 0:C],
                        rhs=ysqb[:, c0:c0 + 512],
                        start=True, stop=True,
                    )
                ny_ps.append(ps)

            # ---------------- row norms of X -> nx2 [128, 2*CH] -------------
            nxps = mmpool.tile([128, 2 * CH], F32, tag="mm")
            for k in range(CH):
                nc.tensor.matmul(
                    nxps[:, 2 * k:2 * k + 2],
                    lhsT=xsqb[:, k * 128:(k + 1) * 128],
                    rhs=ones2b[:],
                    start=True, stop=True,
                )

            # ACT: all Lns grouped, then all Exps (one act-table set each)
            tlns = []
            for h in range(2):
                tln = lnpool.tile([C, HALF], F32, tag="lnt")
                nc.scalar.activation(tln[:], ny_ps[h][:], AF.Ln)
                tlns.append(tln)
            tn = persist.tile([128, CH], F32)
            nc.scalar.activation(
                tn[:], nxps[:].rearrange("p (k two) -> p k two", two=2)[:, :, 0], AF.Ln
            )
            for h in range(2):
                nc.scalar.activation(
                    ry_b[:, h * HALF:(h + 1) * HALF], tlns[h][:], AF.Exp, scale=-0.5
                )
            rx5 = persist.tile([128, CH], F32)
            nc.scalar.activation(rx5[:], tn[:], AF.Exp, bias=ln02[:], scale=-0.5)

            # yhat = ycen * ry  (fp16 x fp16 -> fp16, DVE 2x mode)
            yhatb = persist.tile([C, N], FP16)
            for h in range(2):
                s_ = slice(h * HALF, (h + 1) * HALF)
                nc.vector.tensor_tensor(yhatb[:, s_], ycen16[:, s_], ry_b[:, s_], OP.mult)

            rx5x5 = persist.tile([128, CH], F32)
            nc.vector.tensor_scalar_mul(rx5x5[:], rx5[:], H_BAND)
            nrx5 = persist.tile([128, CH], F32)
            nc.vector.tensor_scalar_mul(nrx5[:], rx5[:], -1.0)

            # ---------------- main loop ----------------
            pmtmp = persist.tile([128, CH], F32)
            pmfin = persist.tile([128, CH], F32)
            aa = persist.tile([128, CH], F32)
            bb = persist.tile([128, CH], F32)
            smax = persist.tile([128, CH], F32)
            den = persist.tile([128, CH], F32)
            rec = persist.tile([128, CH], F32)
            aan = persist.tile([128, CH], F32)
            ssums = persist.tile([128, CH], F32)
            rbs = {}

            for k in range(CH):
                lhs = xcenb[:, k * 128:(k + 1) * 128]
                rb = rbpool.tile([128, N], FP16, tag="rb")
                rbs[k] = rb
                for h in range(2):
                    ps = mmpool.tile([128, HALF], F32, tag="mm")
                    for j in range(4):
                        c0 = h * HALF + j * 512
                        nc.tensor.matmul(
                            ps[:, j * 512:(j + 1) * 512],
                            lhsT=lhs,
                            rhs=yhatb[:, c0:c0 + 512],
                            start=True, stop=True,
                        )
                    # rb = copy(ps); accum = running row-max
                    # (custom-DVE mask-reduce with an all-pass window)
                    init = NEG_BIG if h == 0 else pmtmp[:, k:k + 1]
                    acc = (pmtmp if h == 0 else pmfin)[:, k:k + 1]
                    nc.vector._custom_dve(
                        TENSOR_MASK_REDUCE,
                        out=rb[:, h * HALF:(h + 1) * HALF],
                        in0=ps[:],
                        in1=c3big[:],
                        s0=0.0,
                        s1=init,
                        imm2=1.0,
                        accum_out=acc,
                    )

                if k in (3, 7, 11, 13, 15):
                    # batched per-row stats for chunks g0..k
                    g0 = {3: 0, 7: 4, 11: 8, 13: 12, 15: 14}[k]
                    s_ = slice(g0, k + 1)
                    nc.vector.tensor_tensor(smax[:, s_], pmfin[:, s_], rx5x5[:, s_], OP.mult)
                    nc.vector.tensor_scalar(
                        den[:, s_], smax[:, s_], -1.0, 1.0 + EPS_MIN, OP.mult, OP.add
                    )
                    nc.vector.reciprocal(rec[:, s_], den[:, s_])
                    nc.vector.tensor_tensor(aan[:, s_], rec[:, s_], nrx5[:, s_], OP.mult)
                    nc.vector.tensor_tensor(bb[:, s_], aan[:, s_], pmfin[:, s_], OP.mult)
                    nc.vector.tensor_scalar_mul(aa[:, s_], aan[:, s_], -1.0)

                    for kk in range(g0, k + 1):
                        dummy = dummypool.tile([128, N], FP16, tag="dummy")
                        nc.scalar.activation(
                            dummy[:],
                            rbs.pop(kk)[:],
                            AF.Exp,
                            bias=bb[:, kk:kk + 1],
                            scale=aa[:, kk:kk + 1],
                            accum_out=ssums[:, kk:kk + 1],
                        )

            # ---------------- finalize ----------------
            nc.sync.dma_start(out=out_d[:], in_=ssums[:])

    nc.compile()
    return nc


def _get_nc():
    if "nc" not in _NC_CACHE:
        _NC_CACHE["nc"] = build_nc()
    return _NC_CACHE["nc"]


def make_in_maps(X_features, Y_features):
    X = np.ascontiguousarray(np.asarray(X_features, np.float32).reshape(B, C, N))
    Y = np.ascontiguousarray(np.asarray(Y_features, np.float32).reshape(B, C, N))
    in_maps = []
    for c in range(8):
        b, h = divmod(c, 2)
        in_maps.append({
            "Xh": np.ascontiguousarray(X[b, :, h * NX:(h + 1) * NX]),
            "Yb": Y[b],
        })
    return in_maps


def combine(results):
    """results: list of 8 dicts with 'out' [128, CH] = S per row."""
    out = np.empty(B, np.float32)
    for b in range(B):
        tot = 0.0
        for h in range(2):
            s = results[2 * b + h]["out"].astype(np.float64)
            tot += (1.0 / s).sum()
        out[b] = -np.log(tot / N)
    return out


def kernel(X_features, Y_features):
    nc = _get_nc()
    in_maps = make_in_maps(X_features, Y_features)
    res = run_bass_kernel_spmd(nc, in_maps, core_ids=list(range(8)))
    return combine(res.results)


if __name__ == "__main__":
    rng = np.random.default_rng(0)
    X = rng.standard_normal((B, C, 64, 64)).astype(np.float32)
    Y = rng.standard_normal((B, C, 64, 64)).astype(np.float32)
    print(kernel(X_features=X, Y_features=Y))
